# revision 9
# baseline (speedup 1.0000x reference)
"""BorderLoss Trainium2 kernel.

Reference computation (per element, then global mean):
    loss = max(x,0) - x*y + log1p(exp(-|x|)) == softplus(x) - x*y   (y binary)
    m    = (y > 0)
    ero  = 3x3 min-pool(m)  (OOB ignored / +inf)
    dil  = 3x3 max-pool(m)  (OOB ignored / -inf)
    w    = 1 + (dil - ero)          # RATIO = 2.0
    out  = mean(loss * w)

Device strategy (8 NeuronCores, pure data parallel over batch N=64):
  Each core gets 8 images. Per image:
    - m (=y) and x loaded via SWDGE dtype-casting DMA into bf16 SBUF tiles
      laid out as 4 row-blocks of 128 partitions side-by-side in the free
      dim, with 2 zero pad columns between blocks so horizontal 3-tap sums
      need no edge-case ops.
    - hs = horizontal 3-tap box sum of m        (2 tensor_adds, gpsimd)
    - s  = vertical 3-tap box sum of hs         (tridiagonal matmul on PE,
             plus 1x128-row corner matmuls for cross-block rows, PSUM accum)
      => s = 3x3 box count of ones with out-of-bounds = 0
    - dil = (s >= 0.5);  ero = (s >= cnt - 0.5) where cnt = #in-bounds
      window cells. cnt = rv(p)*cv(j); rv per-partition handled via a
      [128,1] threshold AP, cv==2 columns (j=0,511) fixed by tiny
      edge-correction ops that accumulate separately.
    - sp = softplus(x) on ACT engine; l = sp - x*m (fused STT, accum sum(l))
    - sum(l*dil), sum(l*ero) via fused scalar_tensor_tensor with accum_out.
  Per-core output: [128, n_imgs*12] fp32 partial accumulators.
  Host combines: total = sum(l) + sum(l*dil) - sum(l*ero); mean over all.
"""

import sys
import numpy as np

if "/opt/trn_rl_repo" not in sys.path:
    sys.path.insert(0, "/opt/trn_rl_repo")

H = W = 512
P = 128
NB = 4            # row blocks per image
DBLK = 512        # data cols per block
BLK = DBLK + 2    # block stride in padded layout
LEAD = 2          # leading pad cols
IMG_COLS = LEAD + NB * BLK   # 2058
NACC = 12
N_CORES = 8

_CACHE = {}


def _consts():
    import ml_dtypes
    tri = np.zeros((P, 3 * P), dtype=ml_dtypes.bfloat16)
    for k in range(P):
        tri[k, max(0, k - 1):min(P, k + 2)] = 1.0
    tri[0, P + 127] = 1.0      # U: next block's row 0 -> out row 127
    tri[127, 2 * P + 0] = 1.0  # L: prev block's row 127 -> out row 0
    # thr[:, b]   = t3 for block b : 3*rv - 0.5
    # thr[:, 4+b] = t2 for block b : 2*rv - 0.5
    thr = np.zeros((P, 8), dtype=np.float32)
    for b in range(NB):
        rv = np.full(P, 3.0, dtype=np.float32)
        if b == 0:
            rv[0] = 2.0
        if b == NB - 1:
            rv[P - 1] = 2.0
        thr[:, b] = 3.0 * rv - 0.5
        thr[:, 4 + b] = 2.0 * rv - 0.5
    return tri, thr


def _build(n_imgs):
    import concourse.bass as bass
    import concourse.tile as tile
    from concourse import mybir

    f32 = mybir.dt.float32
    bf16 = mybir.dt.bfloat16
    i32 = mybir.dt.int32
    Alu = mybir.AluOpType
    Act = mybir.ActivationFunctionType

    import concourse.bacc as bacc
    nc = bacc.Bacc(None, target_bir_lowering=False)
    x_d = nc.dram_tensor("x", [n_imgs, H, W], f32, kind="ExternalInput")
    y_d = nc.dram_tensor("y", [n_imgs, H, W], i32, kind="ExternalInput")
    tri_d = nc.dram_tensor("tri", [P, 3 * P], bf16, kind="ExternalInput")
    thr_d = nc.dram_tensor("thr", [P, 8], f32, kind="ExternalInput")
    out_d = nc.dram_tensor("acc", [P, n_imgs * NACC], f32, kind="ExternalOutput")

    def dv(t):
        # [P, IMG_COLS] padded buffer -> [P, NB, DBLK] data view
        return t[:, LEAD:].rearrange("p (b c) -> p b c", c=BLK)[:, :, 0:DBLK]

    with tile.TileContext(nc) as tc:
        with (
            tc.tile_pool(name="consts", bufs=1) as cpool,
            tc.tile_pool(name="io", bufs=2) as io,
            tc.tile_pool(name="work", bufs=2) as work,
            tc.tile_pool(name="accp", bufs=1) as apool,
            tc.tile_pool(name="psum", bufs=8, space=bass.MemorySpace.PSUM) as pp,
        ):
            tri = cpool.tile([P, 3 * P], bf16)
            thr = cpool.tile([P, 8], f32)
            nc.sync.dma_start(tri[:], tri_d[:])
            nc.sync.dma_start(thr[:], thr_d[:])

            accs = apool.tile([P, n_imgs * NACC], f32)

            for i in range(n_imgs):
                m = io.tile([P, IMG_COLS], bf16, tag="m")
                xb = io.tile([P, IMG_COLS], bf16, tag="xb")

                # zero the pad columns of m so shifted reads see zeros
                nc.gpsimd.memset(m[:, 0:LEAD], 0)
                mpads = m[:, LEAD:].rearrange("p (b c) -> p b c", c=BLK)[:, :, DBLK:BLK]
                nc.gpsimd.memset(mpads, 0)

                # casting loads (SWDGE): f32->bf16 and i32->bf16
                nc.gpsimd.dma_start(dv(xb), x_d[i].rearrange("(b p) w -> p b w", p=P))
                nc.gpsimd.dma_start(dv(m), y_d[i].rearrange("(b p) w -> p b w", p=P))

                # horizontal 3-tap box sum (pads make edges exact)
                # h1[c] = m[c] + m[c+1]; hs[c] = h1[c-1] + m[c+1] = 3-tap
                IC = IMG_COLS
                h1 = work.tile([P, IMG_COLS], bf16, tag="h1")
                hs = work.tile([P, IMG_COLS], bf16, tag="hs")
                nc.gpsimd.tensor_add(h1[:, 0:IC - 1], m[:, 0:IC - 1], m[:, 1:IC])
                nc.gpsimd.tensor_add(hs[:, 1:IC - 1], h1[:, 0:IC - 2], m[:, 2:IC])

                # softplus(x) = ln(exp(x) + 1) on ACT (both funcs share the
                # natural_log_exp_and_others table; bias=1 fused into Ln)
                eb = work.tile([P, IMG_COLS], bf16, tag="eb")
                nc.scalar.activation(dv(eb), dv(xb), Act.Exp)
                sp = work.tile([P, IMG_COLS], bf16, tag="sp")
                nc.scalar.activation(dv(sp), dv(eb), Act.Ln, bias=1.0)

                # x*m product
                xm = work.tile([P, IMG_COLS], bf16, tag="xm")
                nc.vector.tensor_mul(dv(xm), dv(xb), dv(m))

                # l = sp - xm, accum Σl ; unpadded [P, NB*DBLK]
                l = work.tile([P, NB * DBLK], bf16, tag="l")
                l3 = l.rearrange("p (b c) -> p b c", c=DBLK)
                a0 = i * NACC
                nc.vector.scalar_tensor_tensor(
                    l3, dv(sp), 0.0, dv(xm), Alu.add, Alu.subtract,
                    accum_out=accs[:, a0:a0 + 1])

                # vertical 3-tap sum via PE; evacuate PSUM->SBUF bf16 on ACT
                s_sb = work.tile([P, NB * DBLK], bf16, tag="s_sb")
                hsv = dv(hs)
                for b in range(NB):
                    s_ps = pp.tile([P, DBLK], f32, tag="s")
                    mms = [(tri[:, 0:P], hsv[:, b, :])]
                    if b > 0:
                        mms.append((tri[:, 2 * P:3 * P], hsv[:, b - 1, :]))
                    if b < NB - 1:
                        mms.append((tri[:, P:2 * P], hsv[:, b + 1, :]))
                    for k, (lt, r) in enumerate(mms):
                        nc.tensor.matmul(s_ps[:], lt, r, start=(k == 0),
                                         stop=(k == len(mms) - 1))
                    nc.scalar.copy(s_sb[:, b * DBLK:(b + 1) * DBLK], s_ps[:])

                # Σ l*dil : dil = (s >= 0.5)
                ld = work.tile([P, NB * DBLK], bf16, tag="ld")
                nc.vector.scalar_tensor_tensor(
                    ld, s_sb[:], 0.5, l[:], Alu.is_ge, Alu.mult,
                    accum_out=accs[:, a0 + 1:a0 + 2])

                # Σ l*ero' per block with per-partition t3 threshold
                le = work.tile([P, NB * DBLK], bf16, tag="le")
                for b in range(NB):
                    sl = slice(b * DBLK, (b + 1) * DBLK)
                    nc.vector.scalar_tensor_tensor(
                        le[:, sl], s_sb[:, sl], thr[:, b:b + 1], l[:, sl],
                        Alu.is_ge, Alu.mult,
                        accum_out=accs[:, a0 + 2 + b:a0 + 3 + b])

                # edge-column corrections (cols 0 and 511 of each block):
                # +Σ l*(s>=t3) (undo wrong threshold), -Σ l*(s>=t2) via host sign
                et = work.tile([P, 16], bf16, tag="et")
                # groups: (accum slot offset, thr col, blocks)
                egroups = [
                    (6, 0, [0]), (7, 1, [1, 2]), (8, 3, [3]),      # t3 (undo)
                    (9, 4, [0]), (10, 5, [1, 2]), (11, 7, [3]),    # t2 (true)
                ]
                eoff = 0
                for slot, tc_col, blks in egroups:
                    if len(blks) == 1:
                        b = blks[0]
                        sap = s_sb[:, b * DBLK:(b + 1) * DBLK:DBLK - 1]
                        lap = l[:, b * DBLK:(b + 1) * DBLK:DBLK - 1]
                        n_e = 2
                    else:
                        b0 = blks[0]
                        sap = s_sb[:, b0 * DBLK:(b0 + 2) * DBLK] \
                            .rearrange("p (b c) -> p b c", c=DBLK)[:, :, ::DBLK - 1]
                        lap = l[:, b0 * DBLK:(b0 + 2) * DBLK] \
                            .rearrange("p (b c) -> p b c", c=DBLK)[:, :, ::DBLK - 1]
                        n_e = 4
                    nc.vector.scalar_tensor_tensor(
                        et[:, eoff:eoff + n_e].rearrange(
                            "p (b c) -> p b c", c=2) if n_e == 4 else et[:, eoff:eoff + n_e],
                        sap, thr[:, tc_col:tc_col + 1], lap,
                        Alu.is_ge, Alu.mult,
                        accum_out=accs[:, a0 + slot:a0 + slot + 1])
                    eoff += n_e

            nc.sync.dma_start(out_d[:], accs[:])

    nc.compile()
    return nc


def _get_nc(n_imgs):
    if n_imgs not in _CACHE:
        _CACHE[n_imgs] = _build(n_imgs)
    return _CACHE[n_imgs]


def _combine(acc, n_imgs):
    # acc: [P, n_imgs*NACC] fp32.
    # total = Σl + Σ(l*dil) - Σ(l*ero), with ero accumulated as:
    #   slots 2..5:  Σ l*(s>=t3) over all cols (t3 wrong at edge cols)
    #   slots 6..8:  Σ l*(s>=t3) on edge cols  (added back to undo)
    #   slots 9..11: Σ l*(s>=t2) on edge cols  (the true edge ero)
    a = acc.reshape(P, n_imgs, NACC).astype(np.float64).sum(axis=0)
    t = (a[:, 0] + a[:, 1] - a[:, 2:6].sum(axis=1)
         + a[:, 6:9].sum(axis=1) - a[:, 9:12].sum(axis=1))
    return t.sum()


def kernel(x, y):
    from concourse import bass_utils

    n = x.shape[0]
    per = n // N_CORES
    nc = _get_nc(per)
    tri, thr = _consts()
    x = np.ascontiguousarray(x, dtype=np.float32)
    y = np.ascontiguousarray(y, dtype=np.int32)
    in_maps = [
        {"x": x[c * per:(c + 1) * per], "y": y[c * per:(c + 1) * per],
         "tri": tri, "thr": thr}
        for c in range(N_CORES)
    ]
    res = bass_utils.run_bass_kernel_spmd(nc, in_maps, core_ids=list(range(N_CORES)))
    total = 0.0
    for r in res.results:
        total += _combine(r["acc"], per)
    return np.float32(total / (n * H * W))


# revision 12
# speedup vs baseline: 1.2306x; 1.2306x over previous
"""BorderLoss Trainium2 kernel.

Reference (per element, then global mean over [64,512,512]):
    loss = softplus(x) - x*y          (y binary {0,1})
    m = (y > 0);  ero = 3x3 min-pool(m);  dil = 3x3 max-pool(m)  (SAME, OOB
    ignored);  w = 1 + (dil - ero);  out = mean(loss * w)

Key identities used:
  * loss = softplus((1-2y)*x)  (stable BCE identity) -> loss is a pure
    activation chain Ln(Exp(z)+1) on the Scalar engine, z = (1-2y)*x.
  * With s = 3x3 box-count of ones (OOB=0) and cnt = #in-bounds cells,
    border = dil-ero = [1 <= s <= cnt-1]. For a row with rv in-bounds
    window rows and interior columns, cnt = 3*rv and
    border <=> |s - mu|/rho <= 1 with mu = 1.5*rv, rho = 1.5*rv - 0.75.
    The tridiagonal vertical-sum matmul is pre-scaled per output row by
    1/rho and shifted by -mu/rho (rank-1 ones term), so the on-chip
    border test is a single |s''| <= 1 tensor-scalar op. Edge columns
    (cv=2) only over-count when s == 2*rv, fixed by one tiny fused op on
    columns {0,511} accumulating sum(l * [s'' >= 0.26]).

Per core (8 images, data parallel across 8 NeuronCores):
  - m = cast-DMA of y (int32->bf16), z = ts(m*-2+1) then cast-DMA of x
    with accum_op=mult (z = (1-2y)*x, no DVE pass for the product).
  - hs = horizontal 3-tap sum: gpsimd add (right neighbor) + SBUF->SBUF
    DMA accumulate (left neighbor).
  - s'' = scaled/shifted vertical 3-tap via PE matmuls into PSUM
    (tridiag variants + U/L cross-block single-entry mats + rank-1 -mu/rho).
  - l = Ln(Exp(z)+1) on ACT with accum_out giving sum(l) free.
  - border = ts(|s''| <= 1) [DVE], lb = l*border [DVE 2x TT],
    sum(lb) via ones-matmul on PE into a persistent PSUM bank.
  - edge fix: one tiny STT per half-image.
Host combines: total = sum(l) + sum(lb) - sum(edge);  mean = total/N/H/W.
"""

import sys
import numpy as np

if "/opt/trn_rl_repo" not in sys.path:
    sys.path.insert(0, "/opt/trn_rl_repo")

H = W = 512
P = 128
NB = 4              # row blocks per image
DBLK = 512
FI = NB * DBLK      # 2048 free cols per image (dense)
NACC = 7            # per img: sum(l), dil h0/h1, ero h0/h1, edge h0/h1
N_CORES = 8
EDGE_THR = 0.26

_CACHE = {}


def _consts():
    import ml_dtypes
    bf = ml_dtypes.bfloat16
    # per-block-type row params: rv (in-bounds window rows) per partition
    rv = np.full((NB, P), 3.0, dtype=np.float64)
    rv[0, 0] = 2.0
    rv[NB - 1, P - 1] = 2.0
    mu = 1.5 * rv                       # [NB, P]
    rho = 1.5 * rv - 0.75
    w = (1.0 / rho)                     # per output-row scale

    tri0 = np.zeros((P, P), dtype=np.float64)
    for k in range(P):
        tri0[k, max(0, k - 1):min(P, k + 2)] = 1.0
    u0 = np.zeros((P, P), dtype=np.float64)
    u0[0, P - 1] = 1.0                  # next block's row 0 -> out row 127
    l0 = np.zeros((P, P), dtype=np.float64)
    l0[P - 1, 0] = 1.0                  # prev block's row 127 -> out row 0

    # tri const [P, 5*P]: T0 scaled for blk0 / mid / blk3, then U, L
    tri = np.zeros((P, 5 * P), dtype=bf)
    for t, b in enumerate((0, 1, NB - 1)):
        tri[:, t * P:(t + 1) * P] = (tri0 * w[b][None, :]).astype(bf)
    tri[:, 3 * P:4 * P] = (u0 * w[1][None, :]).astype(bf)   # target rows rv=3
    tri[:, 4 * P:5 * P] = (l0 * w[1][None, :]).astype(bf)

    # aux const [P, 5*P]: row 0 cols [b*P:(b+1)*P] = -mu/rho for block b;
    # col 4*P.. : ones row [1, DBLK] at row 0; col 0 of cols... use layout:
    #   aux[0, b*P + m] = -mu/rho (blocks 0..3)
    #   aux[:, 4*P:4*P+1] = 1.0 (ones column, lhsT for lb reduction)
    #   aux[0, 4*P+1 : 4*P+1+DBLK] would exceed; use separate region below.
    aux = np.zeros((P, 5 * P + DBLK), dtype=bf)
    for b in range(NB):
        aux[0, b * P:(b + 1) * P] = (-mu[b] / rho[b]).astype(bf)
    aux[:, 4 * P] = bf(1.0)                      # ones column [P,1]
    aux[0, 4 * P + 1:4 * P + 1 + DBLK] = bf(1.0)  # ones row [1, DBLK]
    return tri, aux


def _build(n_imgs):
    import concourse.bass as bass
    import concourse.bacc as bacc
    import concourse.tile as tile
    from concourse import mybir

    f32 = mybir.dt.float32
    bf16 = mybir.dt.bfloat16
    i32 = mybir.dt.int32
    Alu = mybir.AluOpType
    Act = mybir.ActivationFunctionType

    nc = bacc.Bacc(None, target_bir_lowering=False)
    x_d = nc.dram_tensor("x", [n_imgs, H, W], f32, kind="ExternalInput")
    y_d = nc.dram_tensor("y", [n_imgs, H, W], i32, kind="ExternalInput")
    tri_d = nc.dram_tensor("tri", [P, 5 * P], bf16, kind="ExternalInput")
    aux_d = nc.dram_tensor("aux", [P, 5 * P + DBLK], bf16, kind="ExternalInput")
    acc_d = nc.dram_tensor("acc", [P, n_imgs * NACC], f32, kind="ExternalOutput")

    with tile.TileContext(nc) as tc:
        with (
            tc.tile_pool(name="consts", bufs=1) as cpool,
            tc.tile_pool(name="io", bufs=3) as io,
            tc.tile_pool(name="work", bufs=3) as work,
            tc.tile_pool(name="accp", bufs=1) as apool,
            tc.tile_pool(name="ps", bufs=3, space=bass.MemorySpace.PSUM) as pp,
        ):
            tri = cpool.tile([P, 5 * P], bf16)
            aux = cpool.tile([P, 5 * P + DBLK], bf16)
            nc.sync.dma_start(tri[:], tri_d[:])
            nc.sync.dma_start(aux[:], aux_d[:])
            onescol = aux[:, 4 * P:4 * P + 1]          # [P,1] lhsT
            onesrow = aux[0:1, 4 * P + 1:4 * P + 1 + DBLK]  # [1,DBLK] rhs

            accs = apool.tile([P, n_imgs * NACC], f32)

            for i in range(n_imgs):
                a0 = i * NACC
                m = io.tile([P, FI], bf16, tag="m")
                zb = io.tile([P, FI], bf16, tag="zb")
                m3 = m.rearrange("p (b c) -> p b c", c=DBLK)

                # m = cast(y); z = (1-2m)*x  (walrus rejects DMA accum mult,
                # so the product is a 2x TT)
                xb = io.tile([P, FI], bf16, tag="xb")
                nc.gpsimd.dma_start(m3, y_d[i].rearrange("(b p) w -> p b w", p=P))
                nc.gpsimd.dma_start(
                    xb.rearrange("p (b c) -> p b c", c=DBLK),
                    x_d[i].rearrange("(b p) w -> p b w", p=P))
                nc.vector.tensor_scalar(zb[:], m[:], -2.0, 1.0, Alu.mult, Alu.add)
                nc.vector.tensor_mul(zb[:], zb[:], xb[:])

                # horizontal 3-tap box sum (per-block, OOB=0)
                hs = work.tile([P, FI], bf16, tag="hs")
                hs3 = hs.rearrange("p (b c) -> p b c", c=DBLK)
                nc.gpsimd.tensor_add(hs3[:, :, 0:DBLK - 1], m3[:, :, 0:DBLK - 1],
                                     m3[:, :, 1:DBLK])
                nc.gpsimd.tensor_copy(hs3[:, :, DBLK - 1:DBLK],
                                      m3[:, :, DBLK - 1:DBLK])
                nc.gpsimd.dma_start(hs3[:, :, 1:DBLK], m3[:, :, 0:DBLK - 1],
                                    accum_op=Alu.add)

                # loss on ACT: l = Ln(Exp(z)+1), accum -> sum(l)
                eb = work.tile([P, FI], bf16, tag="eb")
                lt = work.tile([P, FI], bf16, tag="lt")
                nc.scalar.activation(eb[:], zb[:], Act.Exp)
                nc.scalar.activation(lt[:], eb[:], Act.Ln, bias=1.0,
                                     accum_out=accs[:, a0:a0 + 1])

                # vertical scaled 3-tap via PE, per half-image (2 banks)
                for h in range(2):
                    sp = pp.tile([P, 2 * DBLK], f32, tag="sp")
                    for j in range(2):
                        b = 2 * h + j
                        tcol = 0 if b == 0 else (2 if b == NB - 1 else 1)
                        o = sp[:, j * DBLK:(j + 1) * DBLK]
                        mms = [(tri[:, tcol * P:(tcol + 1) * P], hs3[:, b, :])]
                        if b > 0:
                            mms.append((tri[:, 4 * P:5 * P], hs3[:, b - 1, :]))
                        if b < NB - 1:
                            mms.append((tri[:, 3 * P:4 * P], hs3[:, b + 1, :]))
                        mms.append((aux[0:1, b * P:(b + 1) * P], onesrow))
                        for k, (ltm, r) in enumerate(mms):
                            nc.tensor.matmul(o, ltm, r, start=(k == 0),
                                             stop=(k == len(mms) - 1))

                    # border = [s'' >= -1.05] - [s'' >= 1.05]; two fused
                    # one-sided products with accumulation (STT is 1x-only,
                    # abs/band ops are ISA-illegal in tensor_scalar)
                    lh = lt[:, h * 2 * DBLK:(h + 1) * 2 * DBLK]
                    u1 = work.tile([P, 2 * DBLK], bf16, tag="u1")
                    nc.vector.scalar_tensor_tensor(
                        u1[:], sp[:], -1.05, lh[:], Alu.is_ge, Alu.mult,
                        accum_out=accs[:, a0 + 1 + h:a0 + 2 + h])
                    u2 = work.tile([P, 2 * DBLK], bf16, tag="u2")
                    nc.vector.scalar_tensor_tensor(
                        u2[:], sp[:], 1.05, lh[:], Alu.is_ge, Alu.mult,
                        accum_out=accs[:, a0 + 3 + h:a0 + 4 + h])
                    # edge-column fix: sum(l * [s'' >= EDGE_THR]) cols {0,511}
                    spe = sp.rearrange("p (b c) -> p b c", c=DBLK)[:, :, ::DBLK - 1]
                    le = lh.rearrange("p (b c) -> p b c", c=DBLK)[:, :, ::DBLK - 1]
                    et = work.tile([P, 4], bf16, tag="et")
                    nc.vector.scalar_tensor_tensor(
                        et.rearrange("p (b c) -> p b c", c=2), spe, EDGE_THR, le,
                        Alu.is_ge, Alu.mult,
                        accum_out=accs[:, a0 + 5 + h:a0 + 6 + h])

            nc.sync.dma_start(acc_d[:], accs[:])

    nc.compile()
    return nc


def _get_nc(n_imgs):
    if n_imgs not in _CACHE:
        _CACHE[n_imgs] = _build(n_imgs)
    return _CACHE[n_imgs]


def _combine(acc, n_imgs):
    # total = sum(l) + sum(l*dil) - sum(l*ero) - sum(edge fix)
    a = acc.reshape(P, n_imgs, NACC).astype(np.float64)
    return (a[:, :, 0].sum() + a[:, :, 1:3].sum() - a[:, :, 3:5].sum()
            - a[:, :, 5:7].sum())


def kernel(x, y):
    from concourse import bass_utils

    n = x.shape[0]
    per = n // N_CORES
    nc = _get_nc(per)
    tri, aux = _consts()
    x = np.ascontiguousarray(x, dtype=np.float32)
    y = np.ascontiguousarray(y, dtype=np.int32)
    in_maps = [
        {"x": x[c * per:(c + 1) * per], "y": y[c * per:(c + 1) * per],
         "tri": tri, "aux": aux}
        for c in range(N_CORES)
    ]
    res = bass_utils.run_bass_kernel_spmd(nc, in_maps, core_ids=list(range(N_CORES)))
    total = 0.0
    for r in res.results:
        total += _combine(r["acc"], per)
    return np.float32(total / (n * H * W))
